# revision 11
# baseline (speedup 1.0000x reference)
"""Trainium2 Bass kernel for nn_SingleConv (gnn_message_passing).

Computes, for each edge e:
  h  = relu(LN(f @ w1.T + b1)); h = relu(LN(h @ w2.T + b2))
  r  = h @ w3.T + b3                      # [E, co*ci*nf]
  out[e, co, do, ci, di] = sum_f r[e, co, ci, f] * basis[e, do, di, f]
returned as [E, 96, 96] fp32.

Sharding: pure data-parallel over E across 8 NeuronCores (2500 edges each).

Per-core kernel structure (128-edge tiles):
  - fp16 MLP: PE transposes + matmuls; LayerNorm via bn_stats with the
    normalize+relu fused into one ScalarE activation (stats in fp32 PSUM).
  - The basis contraction is folded into the third matmul: for each
    (do,di) pair dd,
      out_dd[e, (co,ci)] = sum_{f,h} basis[e,dd,f]*h2[e,h] * w3[(co,ci,f),h]
    i.e. ONE K=96 matmul per dd: lhsT = G_dd.T where
    G_dd[e,(f,h)] = basis[e,dd,f]*h2[e,h], rhs = W3stack[(f,h),(co,ci)]
    (a host-precomputed constant). G for all 9 dd is built by a single
    GpSimd broadcast multiply, transposed per-dd on the PE.
  - PSUM -> SBUF evacuation scatters (co,ci) into the final
    (co*3+do)*96 + ci*3+di layout so the output DMA is contiguous.
  - Software pipelining: tile loads/MLP/G run two tiles ahead of the
    contraction+store so the PE instruction stream never blocks.
"""

import sys

for _p in ("/opt/trn_rl_repo", "/root/.axon_site/_ro/trn_rl_repo"):
    if _p not in sys.path:
        sys.path.insert(0, _p)

import numpy as np

import concourse.bass as bass
import concourse.bacc as bacc
import concourse.tile as tile
from concourse import mybir
from concourse.bass_utils import run_bass_kernel_spmd

E = 20000
N_CORES = 8
EC = E // N_CORES  # 2500 edges per core
P = 128
F_IN = 17  # edge_dim + 1
CH = 32
NF = 3
D = 3  # d_out == d_in == 3
KG = NF * CH  # 96, contraction dim of the fused matmul
RW = CH * CH  # 1024
OUTW = 96 * 96  # 9216
EPS = 1e-5
LOOKAHEAD = 2

AF = mybir.ActivationFunctionType
ALU = mybir.AluOpType
dt = mybir.dt

# (do,di) pairs whose evacuation runs on ScalarE; the rest on VectorE.
ACT_EVAC_DD = (1, 3, 5, 7)


def _expand_ap(ap, dims):
    """Rebuild an AP with explicit free dims [(step, count), ...] (step in
    elements; 0 = broadcast). Keeps the partition dim of `ap`."""
    new = [list(ap.ap[0])] + [[s, c] for s, c in dims]
    return bass.AP(tensor=ap.tensor, offset=ap.offset, ap=new)


def _layernorm_fused(nc, pools, ps_x, e, out_ap):
    """LN over the free dim (32) of psum tile ps_x[:e, :32], fused with relu,
    writing to out_ap. Fast path (gamma==1, beta==0)."""
    stats = pools["stat"].tile([P, 6], dt.float32, tag="stats")
    nc.vector.bn_stats(stats[:e], ps_x[:e])
    mv = pools["stat"].tile([P, 2], dt.float32, tag="mv")
    nc.vector.bn_aggr(mv[:e], stats[:e])
    std = pools["stat"].tile([P, 1], dt.float32, tag="std")
    nc.scalar.activation(std[:e], mv[:e, 1:2], AF.Sqrt, bias=pools["eps"][:e])
    rstd = pools["stat"].tile([P, 1], dt.float32, tag="rstd")
    nc.vector.reciprocal(rstd[:e], std[:e])
    nmr = pools["stat"].tile([P, 1], dt.float32, tag="nmr")
    nc.vector.tensor_scalar(nmr[:e], mv[:e, 0:1], rstd[:e], -1.0, ALU.mult, ALU.mult)
    nc.scalar.activation(out_ap, ps_x[:e], AF.Relu, bias=nmr[:e], scale=rstd[:e])


def build_program(n_edges):
    """Build the per-core Bass program."""
    nc = bacc.Bacc("TRN2", target_bir_lowering=False, debug=False, num_devices=N_CORES)

    f_d = nc.dram_tensor("f16", [n_edges, F_IN], dt.float16, kind="ExternalInput").ap()
    b16_d = nc.dram_tensor("basis16", [n_edges, 27], dt.float16, kind="ExternalInput").ap()
    w1t_d = nc.dram_tensor("w1t", [F_IN, CH], dt.float16, kind="ExternalInput").ap()
    w2t_d = nc.dram_tensor("w2t", [CH, CH], dt.float16, kind="ExternalInput").ap()
    w3s_d = nc.dram_tensor("w3s", [KG, RW], dt.float16, kind="ExternalInput").ap()
    id16_d = nc.dram_tensor("ident16", [P, P], dt.float16, kind="ExternalInput").ap()
    out_d = nc.dram_tensor("out", [n_edges, OUTW], dt.float32, kind="ExternalOutput").ap()

    n_tiles = (n_edges + P - 1) // P

    with tile.TileContext(nc) as tc:
        import contextlib

        with contextlib.ExitStack() as ctx:
            consts = ctx.enter_context(tc.tile_pool(name="consts", bufs=1))
            io_pool = ctx.enter_context(tc.tile_pool(name="io", bufs=2 + LOOKAHEAD))
            mlp_pool = ctx.enter_context(tc.tile_pool(name="mlp", bufs=1 + LOOKAHEAD))
            stat_pool = ctx.enter_context(tc.tile_pool(name="stat", bufs=2 + LOOKAHEAD))
            g_pool = ctx.enter_context(tc.tile_pool(name="g", bufs=1 + LOOKAHEAD))
            out_pool = ctx.enter_context(tc.tile_pool(name="outp", bufs=3))
            ps_small = ctx.enter_context(tc.tile_pool(name="ps_small", bufs=2, space="PSUM"))
            ps_g = ctx.enter_context(tc.tile_pool(name="ps_g", bufs=2, space="PSUM"))
            ps_out = ctx.enter_context(tc.tile_pool(name="ps_out", bufs=2, space="PSUM"))

            pools = {"stat": stat_pool}

            # --- constants (loaded once) ---
            w1t_sb = consts.tile([F_IN, CH], dt.float16)
            nc.sync.dma_start(w1t_sb[:], w1t_d[:])
            w2t_sb = consts.tile([CH, CH], dt.float16)
            nc.sync.dma_start(w2t_sb[:], w2t_d[:])
            w3s_sb = consts.tile([KG, RW], dt.float16)
            nc.sync.dma_start(w3s_sb[:], w3s_d[:])
            id16_sb = consts.tile([P, P], dt.float16)
            nc.sync.dma_start(id16_sb[:], id16_d[:])
            eps_sb = consts.tile([P, 1], dt.float32)
            nc.vector.memset(eps_sb[:], EPS)
            pools["eps"] = eps_sb

            state = {}

            def n_e(it):
                return min(P, n_edges - it * P)

            def emit_front(it):
                """Loads + MLP + scaled-activation build for tile `it`."""
                e0, e = it * P, n_e(it)

                f_sb = io_pool.tile([P, F_IN], dt.float16, tag="f")
                nc.gpsimd.dma_start(f_sb[:e], f_d[e0 : e0 + e])
                b16_sb = io_pool.tile([P, 27], dt.float16, tag="b16")
                nc.gpsimd.dma_start(b16_sb[:e], b16_d[e0 : e0 + e])

                # fT via PE transpose (fp16)
                ps_ft = ps_small.tile([F_IN, P], dt.float16, tag="pss")
                nc.tensor.transpose(ps_ft[:, :e], f_sb[:e, :], id16_sb[:e, :e])
                ft_sb = mlp_pool.tile([F_IN, P], dt.float16, tag="ft")
                nc.scalar.activation(ft_sb[:, :e], ps_ft[:, :e], AF.Copy)

                # layer 1
                ps_h1 = ps_small.tile([P, CH], dt.float32, tag="pss")
                nc.tensor.matmul(ps_h1[:e], ft_sb[:, :e], w1t_sb[:], start=True, stop=True)
                h1n = mlp_pool.tile([P, CH], dt.float16, tag="h1n")
                _layernorm_fused(nc, pools, ps_h1, e, h1n[:e])

                # layer 2
                ps_t1 = ps_small.tile([CH, P], dt.float16, tag="pss")
                nc.tensor.transpose(ps_t1[:, :e], h1n[:e, :], id16_sb[:e, :e])
                h1nT = mlp_pool.tile([CH, P], dt.float16, tag="h1nT")
                nc.scalar.activation(h1nT[:, :e], ps_t1[:, :e], AF.Copy)
                ps_h2 = ps_small.tile([P, CH], dt.float32, tag="pss")
                nc.tensor.matmul(ps_h2[:e], h1nT[:, :e], w2t_sb[:], start=True, stop=True)
                h2n = mlp_pool.tile([P, CH], dt.float16, tag="h2n")
                _layernorm_fused(nc, pools, ps_h2, e, h2n[:e])

                # G[e, dd, f, h] = basis[e, dd*3+f] * h2n[e, h]
                # (one GpSimd op per dd for fine-grained deps), then per-dd
                # PE transpose G_dd [e, 96] -> [96, e], staged in SBUF.
                g_all = mlp_pool.tile([P, D * D * KG], dt.float16, tag="gall")
                g_v = g_all.rearrange("p (t f h) -> p t f h", t=D * D, f=NF)
                gt_sb = g_pool.tile([KG, D * D * P], dt.float16, tag="gt")
                in0 = _expand_ap(h2n[:e, :], [(0, NF), (1, CH)])
                for dd in range(D * D):
                    in1 = _expand_ap(
                        b16_sb[:e, dd * NF : (dd + 1) * NF], [(1, NF), (0, CH)]
                    )
                    nc.gpsimd.tensor_tensor(g_v[:e, dd], in0, in1, ALU.mult)
                    ps_gt = ps_g.tile([KG, P], dt.float16, tag="psg")
                    nc.tensor.transpose(
                        ps_gt[:, :e], g_v[:e, dd, :, :], id16_sb[:e, :e]
                    )
                    nc.scalar.activation(
                        gt_sb[:, dd * P : dd * P + e], ps_gt[:, :e], AF.Copy
                    )
                state[it] = gt_sb

            def emit_back(it):
                """Fused contraction matmuls + evacuation + store for tile `it`."""
                e0, e = it * P, n_e(it)
                gt_sb = state.pop(it)

                out_sb = out_pool.tile([P, OUTW], dt.float32, tag="out")
                out_v = out_sb.rearrange(
                    "p (co d ci q) -> p co d ci q", co=CH, d=D, ci=CH, q=D
                )
                for dd in range(D * D):
                    do_, di_ = divmod(dd, D)
                    ps_o = ps_out.tile([P, RW], dt.float32, tag="pso")
                    for j in range(2):
                        nc.tensor.matmul(
                            ps_o[:e, j * 512 : (j + 1) * 512],
                            gt_sb[:, dd * P : dd * P + e],
                            w3s_sb[:, j * 512 : (j + 1) * 512],
                            start=True, stop=True,
                        )
                    src = ps_o[:e].rearrange("p (co ci) -> p co ci", co=CH)
                    dst = out_v[:e, :, do_, :, di_]
                    if dd in ACT_EVAC_DD:
                        nc.scalar.activation(dst, src, AF.Copy)
                    else:
                        nc.vector.tensor_copy(dst, src)

                for k in range(2):
                    c0 = k * (OUTW // 2)
                    nc.sync.dma_start(
                        out_d[e0 : e0 + e, c0 : c0 + OUTW // 2],
                        out_sb[:e, c0 : c0 + OUTW // 2],
                    )

            for it in range(min(LOOKAHEAD, n_tiles)):
                emit_front(it)
            for it in range(n_tiles):
                if it + LOOKAHEAD < n_tiles:
                    emit_front(it + LOOKAHEAD)
                emit_back(it)

    nc.compile()
    return nc


_CACHE = {}


def _get_program(n_edges):
    if n_edges not in _CACHE:
        _CACHE[n_edges] = build_program(n_edges)
    return _CACHE[n_edges]


def prepare_host_inputs(f, basis, w1, b1, g1, be1, w2, b2, g2, be2, w3, b3):
    """Host-side prep: transpose/cast the small weights, flatten basis, build
    per-core input maps. Only the fast path (zero biases, unit gains) is
    implemented on-device; anything else is rejected loudly."""
    f = np.asarray(f, np.float32)
    basis = np.asarray(basis, np.float32).reshape(E, 27)
    w1 = np.asarray(w1, np.float32)
    w2 = np.asarray(w2, np.float32)
    w3 = np.asarray(w3, np.float32)
    for name, arr, ref in (
        ("b1", b1, 0), ("b2", b2, 0), ("b3", b3, 0),
        ("be1", be1, 0), ("be2", be2, 0), ("g1", g1, 1), ("g2", g2, 1),
    ):
        if np.any(np.asarray(arr, np.float32) != ref):
            raise NotImplementedError(f"non-trivial {name} not supported by this kernel")

    f16 = f.astype(np.float16)
    basis16 = basis.astype(np.float16)
    w1t = np.ascontiguousarray(w1.T).astype(np.float16)  # [17, 32]
    w2t = np.ascontiguousarray(w2.T).astype(np.float16)  # [32, 32]
    # w3 rows are (co, ci, f) flattened; build W3stack[(f,h), (co,ci)]
    w3s = np.ascontiguousarray(
        w3.reshape(CH, CH, NF, CH).transpose(2, 3, 0, 1).reshape(KG, RW)
    ).astype(np.float16)
    id16 = np.eye(P, dtype=np.float16)

    in_maps = []
    for c in range(N_CORES):
        sl = slice(c * EC, (c + 1) * EC)
        in_maps.append(
            {
                "f16": np.ascontiguousarray(f16[sl]),
                "basis16": np.ascontiguousarray(basis16[sl]),
                "w1t": w1t,
                "w2t": w2t,
                "w3s": w3s,
                "ident16": id16,
            }
        )
    return in_maps


def run(inputs, trace=False, **kw):
    in_maps = prepare_host_inputs(**inputs)
    nc = _get_program(EC)
    res = run_bass_kernel_spmd(nc, in_maps, core_ids=list(range(N_CORES)), trace=trace, **kw)
    out = np.concatenate([r["out"].reshape(EC, 96, 96) for r in res.results], axis=0)
    return out, res


def kernel(**inputs) -> np.ndarray:
    out, _ = run(inputs, trace=False)
    return out


if __name__ == "__main__":
    print("building program...")
    nc = _get_program(EC)
    print("built OK")


# revision 21
# speedup vs baseline: 1.2750x; 1.2750x over previous
"""Trainium2 Bass kernel for nn_SingleConv (gnn_message_passing).

Computes, for each edge e:
  h  = relu(LN(f @ w1.T + b1)); h = relu(LN(h @ w2.T + b2))
  r  = h @ w3.T + b3                      # [E, co*ci*nf]
  out[e, co, do, ci, di] = sum_f r[e, co, ci, f] * basis[e, do, di, f]
returned as [E, 96, 96] fp32.

Sharding: pure data-parallel over E across 8 NeuronCores (2500 edges each).

Per-core kernel structure (128-edge tiles):
  - fp16 MLP: PE transposes + matmuls; LayerNorm via bn_stats with the
    normalize+relu fused into one ScalarE activation (stats in fp32 PSUM).
  - The basis contraction is folded into the third matmul: for each
    (do,di) pair dd,
      out_dd[e, (co,ci)] = sum_{f,h} basis[e,dd,f]*h2[e,h] * w3[(co,ci,f),h]
    i.e. ONE K=96 matmul per dd: lhsT = G_dd.T where
    G_dd[e,(f,h)] = basis[e,dd,f]*h2[e,h], rhs = W3stack[(f,h),(co,ci)]
    (a host-precomputed constant). G for all 9 dd is built by a single
    GpSimd broadcast multiply, transposed per-dd on the PE.
  - PSUM -> SBUF evacuation scatters (co,ci) into the final
    (co*3+do)*96 + ci*3+di layout so the output DMA is contiguous.
  - Software pipelining: tile loads/MLP/G run two tiles ahead of the
    contraction+store so the PE instruction stream never blocks.
"""

import sys

for _p in ("/opt/trn_rl_repo", "/root/.axon_site/_ro/trn_rl_repo"):
    if _p not in sys.path:
        sys.path.insert(0, _p)

import numpy as np

import concourse.bass as bass
import concourse.bacc as bacc
import concourse.tile as tile
from concourse import mybir
from concourse.bass_utils import run_bass_kernel_spmd

E = 20000
N_CORES = 8
EC = E // N_CORES  # 2500 edges per core
P = 128
F_IN = 17  # edge_dim + 1
CH = 32
NF = 3
D = 3  # d_out == d_in == 3
KG = NF * CH  # 96, contraction dim of the fused matmul
RW = CH * CH  # 1024
OUTW = 96 * 96  # 9216
EPS = 1e-5
LOOKAHEAD = 2

AF = mybir.ActivationFunctionType
ALU = mybir.AluOpType
dt = mybir.dt

# (do,di) pairs whose evacuation runs on ScalarE; the rest on VectorE.
ACT_EVAC_DD = (0, 2, 4, 6, 8)


def _expand_ap(ap, dims):
    """Rebuild an AP with explicit free dims [(step, count), ...] (step in
    elements; 0 = broadcast). Keeps the partition dim of `ap`."""
    new = [list(ap.ap[0])] + [[s, c] for s, c in dims]
    return bass.AP(tensor=ap.tensor, offset=ap.offset, ap=new)


def _layernorm_fused(nc, pools, ps_x, e, out_ap):
    """LN over the free dim (32) of psum tile ps_x[:e, :32], fused with relu,
    writing to out_ap. Fast path (gamma==1, beta==0)."""
    stats = pools["stat"].tile([P, 6], dt.float32, tag="stats")
    nc.vector.bn_stats(stats[:e], ps_x[:e])
    mv = pools["stat"].tile([P, 2], dt.float32, tag="mv")
    nc.vector.bn_aggr(mv[:e], stats[:e])
    std = pools["stat"].tile([P, 1], dt.float32, tag="std")
    nc.scalar.activation(std[:e], mv[:e, 1:2], AF.Sqrt, bias=pools["eps"][:e])
    rstd = pools["stat"].tile([P, 1], dt.float32, tag="rstd")
    nc.vector.reciprocal(rstd[:e], std[:e])
    nmr = pools["stat"].tile([P, 1], dt.float32, tag="nmr")
    nc.vector.tensor_scalar(nmr[:e], mv[:e, 0:1], rstd[:e], -1.0, ALU.mult, ALU.mult)
    nc.scalar.activation(out_ap, ps_x[:e], AF.Relu, bias=nmr[:e], scale=rstd[:e])


def build_program(n_edges):
    """Build the per-core Bass program."""
    nc = bacc.Bacc("TRN2", target_bir_lowering=False, debug=False, num_devices=N_CORES)

    f_d = nc.dram_tensor("f16", [n_edges, F_IN], dt.float16, kind="ExternalInput").ap()
    b96_d = nc.dram_tensor("b96", [KG, D * D, n_edges], dt.float16, kind="ExternalInput").ap()
    w1t_d = nc.dram_tensor("w1t", [F_IN, CH], dt.float16, kind="ExternalInput").ap()
    w2t_d = nc.dram_tensor("w2t", [CH, CH], dt.float16, kind="ExternalInput").ap()
    w3s_d = nc.dram_tensor("w3s", [KG, RW], dt.float16, kind="ExternalInput").ap()
    id16_d = nc.dram_tensor("ident16", [P, P], dt.float16, kind="ExternalInput").ap()
    out_d = nc.dram_tensor("out", [n_edges, OUTW], dt.float32, kind="ExternalOutput").ap()

    n_tiles = (n_edges + P - 1) // P

    with tile.TileContext(nc) as tc:
        import contextlib

        with contextlib.ExitStack() as ctx:
            consts = ctx.enter_context(tc.tile_pool(name="consts", bufs=1))
            io_pool = ctx.enter_context(tc.tile_pool(name="io", bufs=2 + LOOKAHEAD))
            mlp_pool = ctx.enter_context(tc.tile_pool(name="mlp", bufs=1 + LOOKAHEAD))
            stat_pool = ctx.enter_context(tc.tile_pool(name="stat", bufs=2 + LOOKAHEAD))
            g_pool = ctx.enter_context(tc.tile_pool(name="g", bufs=1 + LOOKAHEAD))
            out_pool = ctx.enter_context(tc.tile_pool(name="outp", bufs=3))
            ps_small = ctx.enter_context(tc.tile_pool(name="ps_small", bufs=2, space="PSUM"))
            ps_out = ctx.enter_context(tc.tile_pool(name="ps_out", bufs=3, space="PSUM"))

            pools = {"stat": stat_pool}

            # --- constants (loaded once) ---
            w1t_sb = consts.tile([F_IN, CH], dt.float16)
            nc.sync.dma_start(w1t_sb[:], w1t_d[:])
            w2t_sb = consts.tile([CH, CH], dt.float16)
            nc.sync.dma_start(w2t_sb[:], w2t_d[:])
            w3s_sb = consts.tile([KG, RW], dt.float16)
            nc.sync.dma_start(w3s_sb[:], w3s_d[:])
            id16_sb = consts.tile([P, P], dt.float16)
            nc.sync.dma_start(id16_sb[:], id16_d[:])
            eps_sb = consts.tile([P, 1], dt.float32)
            nc.vector.memset(eps_sb[:], EPS)
            pools["eps"] = eps_sb

            state = {}

            def n_e(it):
                return min(P, n_edges - it * P)

            def emit_front(it):
                """Loads + MLP + scaled-activation build for tile `it`."""
                e0, e = it * P, n_e(it)

                f_sb = io_pool.tile([P, F_IN], dt.float16, tag="f")
                nc.gpsimd.dma_start(f_sb[:e], f_d[e0 : e0 + e])
                b96_sb = io_pool.tile([KG, D * D, P], dt.float16, tag="b96")
                nc.gpsimd.dma_start(b96_sb[:, :, :e], b96_d[:, :, e0 : e0 + e])

                # fT via PE transpose (fp16)
                ps_ft = ps_small.tile([F_IN, P], dt.float16, tag="pss")
                nc.tensor.transpose(ps_ft[:, :e], f_sb[:e, :], id16_sb[:e, :e])
                ft_sb = mlp_pool.tile([F_IN, P], dt.float16, tag="ft")
                nc.scalar.activation(ft_sb[:, :e], ps_ft[:, :e], AF.Copy)

                # layer 1
                ps_h1 = ps_small.tile([P, CH], dt.float32, tag="pss")
                nc.tensor.matmul(ps_h1[:e], ft_sb[:, :e], w1t_sb[:], start=True, stop=True)
                h1n = mlp_pool.tile([P, CH], dt.float16, tag="h1n")
                _layernorm_fused(nc, pools, ps_h1, e, h1n[:e])

                # layer 2
                ps_t1 = ps_small.tile([CH, P], dt.float16, tag="pss")
                nc.tensor.transpose(ps_t1[:, :e], h1n[:e, :], id16_sb[:e, :e])
                h1nT = mlp_pool.tile([CH, P], dt.float16, tag="h1nT")
                nc.scalar.activation(h1nT[:, :e], ps_t1[:, :e], AF.Copy)
                ps_h2 = ps_small.tile([P, CH], dt.float32, tag="pss")
                nc.tensor.matmul(ps_h2[:e], h1nT[:, :e], w2t_sb[:], start=True, stop=True)
                h2n = mlp_pool.tile([P, CH], dt.float16, tag="h2n")
                _layernorm_fused(nc, pools, ps_h2, e, h2n[:e])

                # h2rep[(f,h), e] = h2n[e, h] for all f: materialize the
                # f-replicated [e, 96] copy (walrus requires a single free dim
                # on matmul stationaries), then transpose via matmul against
                # the identity (exact for fp16 values).
                h2n3 = mlp_pool.tile([P, KG], dt.float16, tag="h2n3")
                nc.gpsimd.tensor_copy(
                    h2n3[:e], _expand_ap(h2n[:e, :], [(0, NF), (1, CH)])
                )
                ps_h2r = ps_small.tile([KG, P], dt.float32, tag="pss")
                nc.tensor.matmul(
                    ps_h2r[:, :e], h2n3[:e], id16_sb[:e, :e], start=True, stop=True
                )
                h2rep = mlp_pool.tile([KG, P], dt.float16, tag="h2rep")
                nc.scalar.activation(h2rep[:, :e], ps_h2r[:, :e], AF.Copy)

                # gt_all[(f,h), dd, e] = h2rep[(f,h), e] * b96[(f,h), dd, e]
                # (one GpSimd op; these are the 9 stationaries of the fused
                # contraction matmuls)
                gt_sb = g_pool.tile([KG, D * D, P], dt.float16, tag="gt")
                in0 = _expand_ap(h2rep[:, :e], [(0, D * D), (1, e)])
                nc.gpsimd.tensor_tensor(gt_sb[:, :, :e], in0, b96_sb[:, :, :e], ALU.mult)
                state[it] = gt_sb

            def emit_back(it):
                """Fused contraction matmuls + evacuation + store for tile `it`."""
                e0, e = it * P, n_e(it)
                gt_sb = state.pop(it)

                out_sb = out_pool.tile([P, OUTW], dt.float32, tag="out")
                out_v = out_sb.rearrange(
                    "p (co d ci q) -> p co d ci q", co=CH, d=D, ci=CH, q=D
                )
                for dd in range(D * D):
                    do_, di_ = divmod(dd, D)
                    ps_o = ps_out.tile([P, RW], dt.float32, tag="pso")
                    for j in range(2):
                        nc.tensor.matmul(
                            ps_o[:e, j * 512 : (j + 1) * 512],
                            gt_sb[:, dd, :e],
                            w3s_sb[:, j * 512 : (j + 1) * 512],
                            start=True, stop=True,
                        )
                    src = ps_o[:e].rearrange("p (co ci) -> p co ci", co=CH)
                    dst = out_v[:e, :, do_, :, di_]
                    if dd in ACT_EVAC_DD:
                        nc.scalar.activation(dst, src, AF.Copy)
                    else:
                        nc.vector.tensor_copy(dst, src)

                for k in range(2):
                    c0 = k * (OUTW // 2)
                    nc.sync.dma_start(
                        out_d[e0 : e0 + e, c0 : c0 + OUTW // 2],
                        out_sb[:e, c0 : c0 + OUTW // 2],
                    )

            for it in range(min(LOOKAHEAD, n_tiles)):
                emit_front(it)
            for it in range(n_tiles):
                if it + LOOKAHEAD < n_tiles:
                    emit_front(it + LOOKAHEAD)
                emit_back(it)

    nc.compile()
    return nc


_CACHE = {}


def _get_program(n_edges):
    if n_edges not in _CACHE:
        _CACHE[n_edges] = build_program(n_edges)
    return _CACHE[n_edges]


def prepare_host_inputs(f, basis, w1, b1, g1, be1, w2, b2, g2, be2, w3, b3):
    """Host-side prep: transpose/cast the small weights, flatten basis, build
    per-core input maps. Only the fast path (zero biases, unit gains) is
    implemented on-device; anything else is rejected loudly."""
    f = np.asarray(f, np.float32)
    basis = np.asarray(basis, np.float32).reshape(E, 27)
    w1 = np.asarray(w1, np.float32)
    w2 = np.asarray(w2, np.float32)
    w3 = np.asarray(w3, np.float32)
    for name, arr, ref in (
        ("b1", b1, 0), ("b2", b2, 0), ("b3", b3, 0),
        ("be1", be1, 0), ("be2", be2, 0), ("g1", g1, 1), ("g2", g2, 1),
    ):
        if np.any(np.asarray(arr, np.float32) != ref):
            raise NotImplementedError(f"non-trivial {name} not supported by this kernel")

    f16 = f.astype(np.float16)
    # B96[(f,h), dd, e] = basis[e, (dd, f)], h-replicated (h is broadcast)
    b96 = np.ascontiguousarray(
        np.broadcast_to(
            basis.reshape(E, D * D, NF).transpose(2, 1, 0)[:, None, :, :],
            (NF, CH, D * D, E),
        ).reshape(KG, D * D, E)
    ).astype(np.float16)
    w1t = np.ascontiguousarray(w1.T).astype(np.float16)  # [17, 32]
    w2t = np.ascontiguousarray(w2.T).astype(np.float16)  # [32, 32]
    # w3 rows are (co, ci, f) flattened; build W3stack[(f,h), (co,ci)]
    w3s = np.ascontiguousarray(
        w3.reshape(CH, CH, NF, CH).transpose(2, 3, 0, 1).reshape(KG, RW)
    ).astype(np.float16)
    id16 = np.eye(P, dtype=np.float16)

    in_maps = []
    for c in range(N_CORES):
        sl = slice(c * EC, (c + 1) * EC)
        in_maps.append(
            {
                "f16": np.ascontiguousarray(f16[sl]),
                "b96": np.ascontiguousarray(b96[:, :, sl]),
                "w1t": w1t,
                "w2t": w2t,
                "w3s": w3s,
                "ident16": id16,
            }
        )
    return in_maps


def run(inputs, trace=False, **kw):
    in_maps = prepare_host_inputs(**inputs)
    nc = _get_program(EC)
    res = run_bass_kernel_spmd(nc, in_maps, core_ids=list(range(N_CORES)), trace=trace, **kw)
    out = np.concatenate([r["out"].reshape(EC, 96, 96) for r in res.results], axis=0)
    return out, res


def kernel(**inputs) -> np.ndarray:
    out, _ = run(inputs, trace=False)
    return out


if __name__ == "__main__":
    print("building program...")
    nc = _get_program(EC)
    print("built OK")


# revision 26
# speedup vs baseline: 1.2922x; 1.0135x over previous
"""Trainium2 Bass kernel for nn_SingleConv (gnn_message_passing).

Computes, for each edge e:
  h  = relu(LN(f @ w1.T + b1)); h = relu(LN(h @ w2.T + b2))
  r  = h @ w3.T + b3                      # [E, co*ci*nf]
  out[e, co, do, ci, di] = sum_f r[e, co, ci, f] * basis[e, do, di, f]
returned as [E, 96, 96] fp32.

Sharding: pure data-parallel over E across 8 NeuronCores (2500 edges each).

Per-core kernel structure (128-edge tiles):
  - fp16 MLP: PE transposes + matmuls; LayerNorm via bn_stats with the
    normalize+relu fused into one ScalarE activation (stats in fp32 PSUM).
  - The basis contraction is folded into the third matmul: for each
    (do,di) pair dd,
      out_dd[e, (co,ci)] = sum_{f,h} basis[e,dd,f]*h2[e,h] * w3[(co,ci,f),h]
    i.e. ONE K=96 matmul per dd: lhsT = G_dd.T where
    G_dd[e,(f,h)] = basis[e,dd,f]*h2[e,h], rhs = W3stack[(f,h),(co,ci)]
    (a host-precomputed constant). G for all 9 dd is built by a single
    GpSimd broadcast multiply, transposed per-dd on the PE.
  - PSUM -> SBUF evacuation scatters (co,ci) into the final
    (co*3+do)*96 + ci*3+di layout so the output DMA is contiguous.
  - Software pipelining: tile loads/MLP/G run two tiles ahead of the
    contraction+store so the PE instruction stream never blocks.
"""

import sys

for _p in ("/opt/trn_rl_repo", "/root/.axon_site/_ro/trn_rl_repo"):
    if _p not in sys.path:
        sys.path.insert(0, _p)

import numpy as np

import concourse.bass as bass
import concourse.bacc as bacc
import concourse.tile as tile
from concourse import mybir
from concourse.bass_utils import run_bass_kernel_spmd

E = 20000
N_CORES = 8
EC = E // N_CORES  # 2500 edges per core
P = 128
F_IN = 17  # edge_dim + 1
CH = 32
NF = 3
D = 3  # d_out == d_in == 3
KG = NF * CH  # 96, contraction dim of the fused matmul
RW = CH * CH  # 1024
OUTW = 96 * 96  # 9216
EPS = 1e-5
LOOKAHEAD = 3

AF = mybir.ActivationFunctionType
ALU = mybir.AluOpType
dt = mybir.dt

# (do,di) pairs whose evacuation runs on ScalarE; the rest on VectorE.
ACT_EVAC_DD = (1, 3, 5, 7)


def _expand_ap(ap, dims):
    """Rebuild an AP with explicit free dims [(step, count), ...] (step in
    elements; 0 = broadcast). Keeps the partition dim of `ap`."""
    new = [list(ap.ap[0])] + [[s, c] for s, c in dims]
    return bass.AP(tensor=ap.tensor, offset=ap.offset, ap=new)


def _layernorm_fused(nc, pools, ps_x, e, out_ap):
    """LN over the free dim (32) of psum tile ps_x[:e, :32], fused with relu,
    writing to out_ap. Fast path (gamma==1, beta==0)."""
    stats = pools["stat"].tile([P, 6], dt.float32, tag="stats")
    nc.vector.bn_stats(stats[:e], ps_x[:e])
    mv = pools["stat"].tile([P, 2], dt.float32, tag="mv")
    nc.vector.bn_aggr(mv[:e], stats[:e])
    std = pools["stat"].tile([P, 1], dt.float32, tag="std")
    nc.scalar.activation(std[:e], mv[:e, 1:2], AF.Sqrt, bias=pools["eps"][:e])
    rstd = pools["stat"].tile([P, 1], dt.float32, tag="rstd")
    nc.vector.reciprocal(rstd[:e], std[:e])
    nmr = pools["stat"].tile([P, 1], dt.float32, tag="nmr")
    nc.vector.tensor_scalar(nmr[:e], mv[:e, 0:1], rstd[:e], -1.0, ALU.mult, ALU.mult)
    nc.scalar.activation(out_ap, ps_x[:e], AF.Relu, bias=nmr[:e], scale=rstd[:e])


def build_program(n_edges):
    """Build the per-core Bass program."""
    nc = bacc.Bacc("TRN2", target_bir_lowering=False, debug=False, num_devices=N_CORES)

    f_d = nc.dram_tensor("f16", [n_edges, F_IN], dt.float16, kind="ExternalInput").ap()
    b96_d = nc.dram_tensor("b96", [KG, D * D, n_edges], dt.float16, kind="ExternalInput").ap()
    w1t_d = nc.dram_tensor("w1t", [F_IN, CH], dt.float16, kind="ExternalInput").ap()
    w2t_d = nc.dram_tensor("w2t", [CH, CH], dt.float16, kind="ExternalInput").ap()
    w3s_d = nc.dram_tensor("w3s", [KG, RW], dt.float16, kind="ExternalInput").ap()
    id16_d = nc.dram_tensor("ident16", [P, P], dt.float16, kind="ExternalInput").ap()
    out_d = nc.dram_tensor("out", [n_edges, OUTW], dt.float32, kind="ExternalOutput").ap()

    n_tiles = (n_edges + P - 1) // P

    with tile.TileContext(nc) as tc:
        import contextlib

        with contextlib.ExitStack() as ctx:
            consts = ctx.enter_context(tc.tile_pool(name="consts", bufs=1))
            io_pool = ctx.enter_context(tc.tile_pool(name="io", bufs=2 + LOOKAHEAD))
            mlp_pool = ctx.enter_context(tc.tile_pool(name="mlp", bufs=1 + LOOKAHEAD))
            stat_pool = ctx.enter_context(tc.tile_pool(name="stat", bufs=2 + LOOKAHEAD))
            g_pool = ctx.enter_context(tc.tile_pool(name="g", bufs=1 + LOOKAHEAD))
            out_pool = ctx.enter_context(tc.tile_pool(name="outp", bufs=4))
            ps_small = ctx.enter_context(tc.tile_pool(name="ps_small", bufs=2, space="PSUM"))
            ps_out = ctx.enter_context(tc.tile_pool(name="ps_out", bufs=3, space="PSUM"))

            pools = {"stat": stat_pool}

            # --- constants (loaded once) ---
            w1t_sb = consts.tile([F_IN, CH], dt.float16)
            nc.sync.dma_start(w1t_sb[:], w1t_d[:])
            w2t_sb = consts.tile([CH, CH], dt.float16)
            nc.sync.dma_start(w2t_sb[:], w2t_d[:])
            w3s_sb = consts.tile([KG, RW], dt.float16)
            nc.sync.dma_start(w3s_sb[:], w3s_d[:])
            id16_sb = consts.tile([P, P], dt.float16)
            nc.sync.dma_start(id16_sb[:], id16_d[:])
            eps_sb = consts.tile([P, 1], dt.float32)
            nc.vector.memset(eps_sb[:], EPS)
            pools["eps"] = eps_sb

            state = {}

            def n_e(it):
                return min(P, n_edges - it * P)

            def emit_front(it):
                """Loads + MLP + scaled-activation build for tile `it`."""
                e0, e = it * P, n_e(it)

                f_sb = io_pool.tile([P, F_IN], dt.float16, tag="f")
                nc.gpsimd.dma_start(f_sb[:e], f_d[e0 : e0 + e])
                b96_sb = io_pool.tile([KG, D * D, P], dt.float16, tag="b96")
                nc.gpsimd.dma_start(b96_sb[:, :, :e], b96_d[:, :, e0 : e0 + e])

                # fT via PE transpose (fp16)
                ps_ft = ps_small.tile([F_IN, P], dt.float16, tag="pss")
                nc.tensor.transpose(ps_ft[:, :e], f_sb[:e, :], id16_sb[:e, :e])
                ft_sb = mlp_pool.tile([F_IN, P], dt.float16, tag="ft")
                nc.vector.tensor_copy(ft_sb[:, :e], ps_ft[:, :e])

                # layer 1
                ps_h1 = ps_small.tile([P, CH], dt.float32, tag="pss")
                nc.tensor.matmul(ps_h1[:e], ft_sb[:, :e], w1t_sb[:], start=True, stop=True)
                h1n = mlp_pool.tile([P, CH], dt.float16, tag="h1n")
                _layernorm_fused(nc, pools, ps_h1, e, h1n[:e])

                # layer 2
                ps_t1 = ps_small.tile([CH, P], dt.float16, tag="pss")
                nc.tensor.transpose(ps_t1[:, :e], h1n[:e, :], id16_sb[:e, :e])
                h1nT = mlp_pool.tile([CH, P], dt.float16, tag="h1nT")
                nc.vector.tensor_copy(h1nT[:, :e], ps_t1[:, :e])
                ps_h2 = ps_small.tile([P, CH], dt.float32, tag="pss")
                nc.tensor.matmul(ps_h2[:e], h1nT[:, :e], w2t_sb[:], start=True, stop=True)
                h2n = mlp_pool.tile([P, CH], dt.float16, tag="h2n")
                _layernorm_fused(nc, pools, ps_h2, e, h2n[:e])

                # h2rep[(f,h), e] = h2n[e, h] for all f: materialize the
                # f-replicated [e, 96] copy (walrus requires a single free dim
                # on matmul stationaries), then transpose via matmul against
                # the identity (exact for fp16 values).
                h2n3 = mlp_pool.tile([P, KG], dt.float16, tag="h2n3")
                nc.gpsimd.tensor_copy(
                    h2n3[:e], _expand_ap(h2n[:e, :], [(0, NF), (1, CH)])
                )
                ps_h2r = ps_small.tile([KG, P], dt.float32, tag="pss")
                nc.tensor.matmul(
                    ps_h2r[:, :e], h2n3[:e], id16_sb[:e, :e], start=True, stop=True
                )
                h2rep = mlp_pool.tile([KG, P], dt.float16, tag="h2rep")
                nc.scalar.activation(h2rep[:, :e], ps_h2r[:, :e], AF.Copy)

                # gt_all[(f,h), dd, e] = h2rep[(f,h), e] * b96[(f,h), dd, e]
                # (one GpSimd op; these are the 9 stationaries of the fused
                # contraction matmuls)
                gt_sb = g_pool.tile([KG, D * D, P], dt.float16, tag="gt")
                in0 = _expand_ap(h2rep[:, :e], [(0, D * D), (1, e)])
                nc.gpsimd.tensor_tensor(gt_sb[:, :, :e], in0, b96_sb[:, :, :e], ALU.mult)
                state[it] = gt_sb

            def emit_back(it):
                """Fused contraction matmuls + evacuation + store for tile `it`."""
                e0, e = it * P, n_e(it)
                gt_sb = state.pop(it)

                out_sb = out_pool.tile([P, OUTW], dt.float32, tag="out")
                out_v = out_sb.rearrange(
                    "p (co d ci q) -> p co d ci q", co=CH, d=D, ci=CH, q=D
                )
                for dd in range(D * D):
                    do_, di_ = divmod(dd, D)
                    ps_o = ps_out.tile([P, RW], dt.float32, tag="pso")
                    for j in range(2):
                        nc.tensor.matmul(
                            ps_o[:e, j * 512 : (j + 1) * 512],
                            gt_sb[:, dd, :e],
                            w3s_sb[:, j * 512 : (j + 1) * 512],
                            start=True, stop=True,
                        )
                    src = ps_o[:e].rearrange("p (co ci) -> p co ci", co=CH)
                    dst = out_v[:e, :, do_, :, di_]
                    if dd in ACT_EVAC_DD:
                        nc.scalar.activation(dst, src, AF.Copy)
                    else:
                        nc.vector.tensor_copy(dst, src)

                for k in range(2):
                    c0 = k * (OUTW // 2)
                    nc.sync.dma_start(
                        out_d[e0 : e0 + e, c0 : c0 + OUTW // 2],
                        out_sb[:e, c0 : c0 + OUTW // 2],
                    )

            for it in range(min(LOOKAHEAD, n_tiles)):
                emit_front(it)
            for it in range(n_tiles):
                if it + LOOKAHEAD < n_tiles:
                    emit_front(it + LOOKAHEAD)
                emit_back(it)

    nc.compile()
    return nc


_CACHE = {}


def _get_program(n_edges):
    if n_edges not in _CACHE:
        _CACHE[n_edges] = build_program(n_edges)
    return _CACHE[n_edges]


def prepare_host_inputs(f, basis, w1, b1, g1, be1, w2, b2, g2, be2, w3, b3):
    """Host-side prep: transpose/cast the small weights, flatten basis, build
    per-core input maps. Only the fast path (zero biases, unit gains) is
    implemented on-device; anything else is rejected loudly."""
    f = np.asarray(f, np.float32)
    basis = np.asarray(basis, np.float32).reshape(E, 27)
    w1 = np.asarray(w1, np.float32)
    w2 = np.asarray(w2, np.float32)
    w3 = np.asarray(w3, np.float32)
    for name, arr, ref in (
        ("b1", b1, 0), ("b2", b2, 0), ("b3", b3, 0),
        ("be1", be1, 0), ("be2", be2, 0), ("g1", g1, 1), ("g2", g2, 1),
    ):
        if np.any(np.asarray(arr, np.float32) != ref):
            raise NotImplementedError(f"non-trivial {name} not supported by this kernel")

    f16 = f.astype(np.float16)
    # B96[(f,h), dd, e] = basis[e, (dd, f)], h-replicated (h is broadcast)
    b96 = np.ascontiguousarray(
        np.broadcast_to(
            basis.reshape(E, D * D, NF).transpose(2, 1, 0)[:, None, :, :],
            (NF, CH, D * D, E),
        ).reshape(KG, D * D, E)
    ).astype(np.float16)
    w1t = np.ascontiguousarray(w1.T).astype(np.float16)  # [17, 32]
    w2t = np.ascontiguousarray(w2.T).astype(np.float16)  # [32, 32]
    # w3 rows are (co, ci, f) flattened; build W3stack[(f,h), (co,ci)]
    w3s = np.ascontiguousarray(
        w3.reshape(CH, CH, NF, CH).transpose(2, 3, 0, 1).reshape(KG, RW)
    ).astype(np.float16)
    id16 = np.eye(P, dtype=np.float16)

    in_maps = []
    for c in range(N_CORES):
        sl = slice(c * EC, (c + 1) * EC)
        in_maps.append(
            {
                "f16": np.ascontiguousarray(f16[sl]),
                "b96": np.ascontiguousarray(b96[:, :, sl]),
                "w1t": w1t,
                "w2t": w2t,
                "w3s": w3s,
                "ident16": id16,
            }
        )
    return in_maps


def run(inputs, trace=False, **kw):
    in_maps = prepare_host_inputs(**inputs)
    nc = _get_program(EC)
    res = run_bass_kernel_spmd(nc, in_maps, core_ids=list(range(N_CORES)), trace=trace, **kw)
    out = np.concatenate([r["out"].reshape(EC, 96, 96) for r in res.results], axis=0)
    return out, res


def kernel(**inputs) -> np.ndarray:
    out, _ = run(inputs, trace=False)
    return out


if __name__ == "__main__":
    print("building program...")
    nc = _get_program(EC)
    print("built OK")


# revision 28
# speedup vs baseline: 1.3340x; 1.0324x over previous
"""Trainium2 Bass kernel for nn_SingleConv (gnn_message_passing).

Computes, for each edge e:
  h  = relu(LN(f @ w1.T + b1)); h = relu(LN(h @ w2.T + b2))
  r  = h @ w3.T + b3                      # [E, co*ci*nf]
  out[e, co, do, ci, di] = sum_f r[e, co, ci, f] * basis[e, do, di, f]
returned as [E, 96, 96] fp32.

Sharding: pure data-parallel over E across 8 NeuronCores (2500 edges each).

Per-core kernel structure (128-edge tiles):
  - fp16 MLP: PE transposes + matmuls; LayerNorm via bn_stats with the
    normalize+relu fused into one ScalarE activation (stats in fp32 PSUM).
  - The basis contraction is folded into the third matmul: for each
    (do,di) pair dd,
      out_dd[e, (co,ci)] = sum_{f,h} basis[e,dd,f]*h2[e,h] * w3[(co,ci,f),h]
    i.e. ONE K=96 matmul per dd: lhsT = G_dd.T where
    G_dd[e,(f,h)] = basis[e,dd,f]*h2[e,h], rhs = W3stack[(f,h),(co,ci)]
    (a host-precomputed constant). G for all 9 dd is built by a single
    GpSimd broadcast multiply, transposed per-dd on the PE.
  - PSUM -> SBUF evacuation scatters (co,ci) into the final
    (co*3+do)*96 + ci*3+di layout so the output DMA is contiguous.
  - Software pipelining: tile loads/MLP/G run two tiles ahead of the
    contraction+store so the PE instruction stream never blocks.
"""

import sys

for _p in ("/opt/trn_rl_repo", "/root/.axon_site/_ro/trn_rl_repo"):
    if _p not in sys.path:
        sys.path.insert(0, _p)

import numpy as np

import concourse.bass as bass
import concourse.bacc as bacc
import concourse.tile as tile
from concourse import mybir
from concourse.bass_utils import run_bass_kernel_spmd

E = 20000
N_CORES = 8
EC = E // N_CORES  # 2500 edges per core
P = 128
F_IN = 17  # edge_dim + 1
CH = 32
NF = 3
D = 3  # d_out == d_in == 3
KG = NF * CH  # 96, contraction dim of the fused matmul
RW = CH * CH  # 1024
OUTW = 96 * 96  # 9216
EPS = 1e-5
LOOKAHEAD = 3

AF = mybir.ActivationFunctionType
ALU = mybir.AluOpType
dt = mybir.dt

# (do,di) pairs whose evacuation runs on ScalarE; the rest on VectorE.
ACT_EVAC_DD = (1, 3, 5, 7)


def _expand_ap(ap, dims):
    """Rebuild an AP with explicit free dims [(step, count), ...] (step in
    elements; 0 = broadcast). Keeps the partition dim of `ap`."""
    new = [list(ap.ap[0])] + [[s, c] for s, c in dims]
    return bass.AP(tensor=ap.tensor, offset=ap.offset, ap=new)


def _layernorm_fused(nc, pools, ps_x, e, out_ap):
    """LN over the free dim (32) of psum tile ps_x[:e, :32], fused with relu,
    writing to out_ap. Fast path (gamma==1, beta==0)."""
    stats = pools["stat"].tile([P, 6], dt.float32, tag="stats")
    nc.vector.bn_stats(stats[:e], ps_x[:e])
    mv = pools["stat"].tile([P, 2], dt.float32, tag="mv")
    nc.vector.bn_aggr(mv[:e], stats[:e])
    std = pools["stat"].tile([P, 1], dt.float32, tag="std")
    nc.scalar.activation(std[:e], mv[:e, 1:2], AF.Sqrt, bias=pools["eps"][:e])
    rstd = pools["stat"].tile([P, 1], dt.float32, tag="rstd")
    nc.vector.reciprocal(rstd[:e], std[:e])
    nmr = pools["stat"].tile([P, 1], dt.float32, tag="nmr")
    nc.vector.tensor_scalar(nmr[:e], mv[:e, 0:1], rstd[:e], -1.0, ALU.mult, ALU.mult)
    nc.scalar.activation(out_ap, ps_x[:e], AF.Relu, bias=nmr[:e], scale=rstd[:e])


def build_program(n_edges):
    """Build the per-core Bass program."""
    nc = bacc.Bacc("TRN2", target_bir_lowering=False, debug=False, num_devices=N_CORES)

    f_d = nc.dram_tensor("f16", [n_edges, F_IN], dt.float16, kind="ExternalInput").ap()
    b96_d = nc.dram_tensor("b96", [KG, D * D, n_edges], dt.float16, kind="ExternalInput").ap()
    w1t_d = nc.dram_tensor("w1t", [F_IN, CH], dt.float16, kind="ExternalInput").ap()
    w2t_d = nc.dram_tensor("w2t", [CH, CH], dt.float16, kind="ExternalInput").ap()
    w3s_d = nc.dram_tensor("w3s", [KG, RW], dt.float16, kind="ExternalInput").ap()
    id16_d = nc.dram_tensor("ident16", [P, P], dt.float16, kind="ExternalInput").ap()
    out_d = nc.dram_tensor("out", [n_edges, OUTW], dt.float32, kind="ExternalOutput").ap()

    n_tiles = (n_edges + P - 1) // P

    with tile.TileContext(nc) as tc:
        import contextlib

        with contextlib.ExitStack() as ctx:
            consts = ctx.enter_context(tc.tile_pool(name="consts", bufs=1))
            io_pool = ctx.enter_context(tc.tile_pool(name="io", bufs=2 + LOOKAHEAD))
            mlp_pool = ctx.enter_context(tc.tile_pool(name="mlp", bufs=1 + LOOKAHEAD))
            stat_pool = ctx.enter_context(tc.tile_pool(name="stat", bufs=2 + LOOKAHEAD))
            g_pool = ctx.enter_context(tc.tile_pool(name="g", bufs=1 + LOOKAHEAD))
            out_pool = ctx.enter_context(tc.tile_pool(name="outp", bufs=4))
            ps_small = ctx.enter_context(tc.tile_pool(name="ps_small", bufs=2, space="PSUM"))
            ps_out = ctx.enter_context(tc.tile_pool(name="ps_out", bufs=3, space="PSUM"))

            pools = {"stat": stat_pool}

            # --- constants (loaded once) ---
            w1t_sb = consts.tile([F_IN, CH], dt.float16)
            nc.sync.dma_start(w1t_sb[:], w1t_d[:])
            w2t_sb = consts.tile([CH, CH], dt.float16)
            nc.sync.dma_start(w2t_sb[:], w2t_d[:])
            w3s_sb = consts.tile([KG, RW], dt.float16)
            nc.sync.dma_start(w3s_sb[:], w3s_d[:])
            id16_sb = consts.tile([P, P], dt.float16)
            nc.sync.dma_start(id16_sb[:], id16_d[:])
            eps_sb = consts.tile([P, 1], dt.float32)
            nc.vector.memset(eps_sb[:], EPS)
            pools["eps"] = eps_sb

            state = {}

            def n_e(it):
                return min(P, n_edges - it * P)

            def emit_front(it):
                """Loads + MLP + scaled-activation build for tile `it`."""
                e0, e = it * P, n_e(it)

                f_sb = io_pool.tile([P, F_IN], dt.float16, tag="f")
                nc.gpsimd.dma_start(f_sb[:e], f_d[e0 : e0 + e])
                b96_sb = io_pool.tile([KG, D * D, P], dt.float16, tag="b96")
                nc.gpsimd.dma_start(b96_sb[:, :, :e], b96_d[:, :, e0 : e0 + e])

                # fT via PE transpose (fp16)
                ps_ft = ps_small.tile([F_IN, P], dt.float16, tag="pss")
                nc.tensor.transpose(ps_ft[:, :e], f_sb[:e, :], id16_sb[:e, :e])
                ft_sb = mlp_pool.tile([F_IN, P], dt.float16, tag="ft")
                nc.vector.tensor_copy(ft_sb[:, :e], ps_ft[:, :e])

                # layer 1
                ps_h1 = ps_small.tile([P, CH], dt.float32, tag="pss")
                nc.tensor.matmul(ps_h1[:e], ft_sb[:, :e], w1t_sb[:], start=True, stop=True)
                h1n = mlp_pool.tile([P, CH], dt.float16, tag="h1n")
                _layernorm_fused(nc, pools, ps_h1, e, h1n[:e])

                # layer 2
                ps_t1 = ps_small.tile([CH, P], dt.float16, tag="pss")
                nc.tensor.transpose(ps_t1[:, :e], h1n[:e, :], id16_sb[:e, :e])
                h1nT = mlp_pool.tile([CH, P], dt.float16, tag="h1nT")
                nc.vector.tensor_copy(h1nT[:, :e], ps_t1[:, :e])
                ps_h2 = ps_small.tile([P, CH], dt.float32, tag="pss")
                nc.tensor.matmul(ps_h2[:e], h1nT[:, :e], w2t_sb[:], start=True, stop=True)
                h2n = mlp_pool.tile([P, CH], dt.float16, tag="h2n")
                _layernorm_fused(nc, pools, ps_h2, e, h2n[:e])

                # h2rep[(f,h), e] = h2n[e, h] for all f: materialize the
                # f-replicated [e, 96] copy (walrus requires a single free dim
                # on matmul stationaries), then transpose via matmul against
                # the identity (exact for fp16 values).
                h2n3 = mlp_pool.tile([P, KG], dt.float16, tag="h2n3")
                nc.gpsimd.tensor_copy(
                    h2n3[:e], _expand_ap(h2n[:e, :], [(0, NF), (1, CH)])
                )
                ps_h2r = ps_small.tile([KG, P], dt.float32, tag="pss")
                nc.tensor.matmul(
                    ps_h2r[:, :e], h2n3[:e], id16_sb[:e, :e], start=True, stop=True
                )
                h2rep = mlp_pool.tile([KG, P], dt.float16, tag="h2rep")
                nc.scalar.activation(h2rep[:, :e], ps_h2r[:, :e], AF.Copy)

                # gt_all[(f,h), dd, e] = h2rep[(f,h), e] * b96[(f,h), dd, e]
                # (one GpSimd op; these are the 9 stationaries of the fused
                # contraction matmuls)
                gt_sb = g_pool.tile([KG, D * D, P], dt.float16, tag="gt")
                in0 = _expand_ap(h2rep[:, :e], [(0, D * D), (1, e)])
                nc.gpsimd.tensor_tensor(gt_sb[:, :, :e], in0, b96_sb[:, :, :e], ALU.mult)
                state[it] = gt_sb

            def emit_back(it):
                """Fused contraction matmuls + evacuation + store for tile `it`."""
                e0, e = it * P, n_e(it)
                gt_sb = state.pop(it)

                out_sb = out_pool.tile([P, OUTW], dt.float32, tag="out")
                out_v = out_sb.rearrange(
                    "p (co d ci q) -> p co d ci q", co=CH, d=D, ci=CH, q=D
                )
                for dd in range(D * D):
                    do_, di_ = divmod(dd, D)
                    ps_o = ps_out.tile([P, RW], dt.float32, tag="pso")
                    for j in range(2):
                        nc.tensor.matmul(
                            ps_o[:e, j * 512 : (j + 1) * 512],
                            gt_sb[:, dd, :e],
                            w3s_sb[:, j * 512 : (j + 1) * 512],
                            start=True, stop=True,
                        )
                    src = ps_o[:e].rearrange("p (co ci) -> p co ci", co=CH)
                    dst = out_v[:e, :, do_, :, di_]
                    if dd in ACT_EVAC_DD:
                        nc.scalar.activation(dst, src, AF.Copy)
                    else:
                        nc.vector.tensor_copy(dst, src)

                for k in range(2):
                    c0 = k * (OUTW // 2)
                    nc.sync.dma_start(
                        out_d[e0 : e0 + e, c0 : c0 + OUTW // 2],
                        out_sb[:e, c0 : c0 + OUTW // 2],
                    )

            for it in range(min(LOOKAHEAD, n_tiles)):
                emit_front(it)
            for it in range(n_tiles):
                if it + LOOKAHEAD < n_tiles:
                    emit_front(it + LOOKAHEAD)
                emit_back(it)

    nc.compile()
    return nc


_CACHE = {}


def _get_program(n_edges):
    if n_edges not in _CACHE:
        _CACHE[n_edges] = build_program(n_edges)
    return _CACHE[n_edges]


def prepare_host_inputs(f, basis, w1, b1, g1, be1, w2, b2, g2, be2, w3, b3):
    """Host-side prep: transpose/cast the small weights, flatten basis, build
    per-core input maps. Only the fast path (zero biases, unit gains) is
    implemented on-device; anything else is rejected loudly."""
    f = np.asarray(f, np.float32)
    basis = np.asarray(basis, np.float32).reshape(E, 27)
    w1 = np.asarray(w1, np.float32)
    w2 = np.asarray(w2, np.float32)
    w3 = np.asarray(w3, np.float32)
    for name, arr, ref in (
        ("b1", b1, 0), ("b2", b2, 0), ("b3", b3, 0),
        ("be1", be1, 0), ("be2", be2, 0), ("g1", g1, 1), ("g2", g2, 1),
    ):
        if np.any(np.asarray(arr, np.float32) != ref):
            raise NotImplementedError(f"non-trivial {name} not supported by this kernel")

    f16 = f.astype(np.float16)
    # B96[(f,h), dd, e] = basis[e, (dd, f)], h-replicated (h is broadcast)
    b96 = np.ascontiguousarray(
        np.broadcast_to(
            basis.reshape(E, D * D, NF).transpose(2, 1, 0)[:, None, :, :],
            (NF, CH, D * D, E),
        ).reshape(KG, D * D, E)
    ).astype(np.float16)
    w1t = np.ascontiguousarray(w1.T).astype(np.float16)  # [17, 32]
    w2t = np.ascontiguousarray(w2.T).astype(np.float16)  # [32, 32]
    # w3 rows are (co, ci, f) flattened; build W3stack[(f,h), (co,ci)]
    w3s = np.ascontiguousarray(
        w3.reshape(CH, CH, NF, CH).transpose(2, 3, 0, 1).reshape(KG, RW)
    ).astype(np.float16)
    id16 = np.eye(P, dtype=np.float16)

    in_maps = []
    for c in range(N_CORES):
        sl = slice(c * EC, (c + 1) * EC)
        in_maps.append(
            {
                "f16": np.ascontiguousarray(f16[sl]),
                "b96": np.ascontiguousarray(b96[:, :, sl]),
                "w1t": w1t,
                "w2t": w2t,
                "w3s": w3s,
                "ident16": id16,
            }
        )
    return in_maps


def run(inputs, trace=False, **kw):
    in_maps = prepare_host_inputs(**inputs)
    nc = _get_program(EC)
    res = run_bass_kernel_spmd(nc, in_maps, core_ids=list(range(N_CORES)), trace=trace, **kw)
    out = np.concatenate([r["out"].reshape(EC, 96, 96) for r in res.results], axis=0)
    return out, res


def kernel(**inputs) -> np.ndarray:
    out, _ = run(inputs, trace=False)
    return out


if __name__ == "__main__":
    print("building program...")
    nc = _get_program(EC)
    print("built OK")
